# revision 19
# baseline (speedup 1.0000x reference)
"""Trainium2 Bass kernel for a 6-layer dense transformer encoder.

Sharding: 8 cores = 2 batch rows x 4 token-blocks (512 tokens each).
Activations are kept feature-major ([D partitions, S free]).  Each core
holds its full 2048-token row of h in bf16 (for K/V projections) plus its
own 512-token shard (for Q / residual stream).  A 4-way AllGather per
layer rebuilds the full row from the per-core layer outputs.

Compute: bf16 matmuls with f32 PSUM accumulation; LayerNorm statistics
via ones-vector matmuls on the tensor engine (partition-axis reductions);
softmax without max-subtraction (logits provably tiny); softmax
denominator folded into the A@V matmul through a ones-column appended to
the transposed V tiles.
"""

import os
import numpy as np
import ml_dtypes
from contextlib import ExitStack

import concourse.bass as bass
import concourse.bacc as bacc
import concourse.tile as tile
import concourse.mybir as mybir
from concourse.bass_utils import run_bass_kernel_spmd
from concourse.masks import make_identity

F32 = mybir.dt.float32
BF16 = mybir.dt.bfloat16
I32 = mybir.dt.int32
AF = mybir.ActivationFunctionType
ALU = mybir.AluOpType

VOCAB, MAXLEN, NLAYERS, D, H, DK, DFF = 32000, 2048, 6, 1024, 16, 64, 4096
B, S = 2, 2048
P = 128
ND = D // P          # 8 d-chunks
NF = DFF // P        # 32 f-chunks
NS = S // P          # 16 s-chunks per row
SB = 512             # tokens per core
NSB = SB // P        # 4 s-chunks per core shard
NCORES = 8
GROUPS = [[0, 1, 2, 3], [4, 5, 6, 7]]
LN_EPS = 1e-5
SCALE = 1.0 / 8.0    # 1/sqrt(DK)


def build_nc(n_layers=NLAYERS, n_cores=NCORES):
    nc = bacc.Bacc(None, target_bir_lowering=False, num_devices=n_cores)
    L = n_layers
    use_ag = n_cores > 1

    emb = nc.dram_tensor("emb", [VOCAB, D], F32, kind="ExternalInput")
    idx = nc.dram_tensor("idx", [S], I32, kind="ExternalInput")
    idxm = nc.dram_tensor("idxm", [SB], I32, kind="ExternalInput")
    posT = nc.dram_tensor("posT", [D, S], F32, kind="ExternalInput")
    posTm = nc.dram_tensor("posTm", [D, SB], F32, kind="ExternalInput")
    wqT = nc.dram_tensor("wqT", [L, P, P], BF16, kind="ExternalInput")
    wkT = nc.dram_tensor("wkT", [L, P, P], BF16, kind="ExternalInput")
    wvT = nc.dram_tensor("wvT", [L, DK, DK], BF16, kind="ExternalInput")
    woT = nc.dram_tensor("woT", [L, D, D], BF16, kind="ExternalInput")
    w1T = nc.dram_tensor("w1T", [L, D, DFF], BF16, kind="ExternalInput")
    w2T = nc.dram_tensor("w2T", [L, DFF, D], BF16, kind="ExternalInput")
    bo = nc.dram_tensor("bo", [L, D], F32, kind="ExternalInput")
    b1 = nc.dram_tensor("b1", [L, DFF], F32, kind="ExternalInput")
    b2 = nc.dram_tensor("b2", [L, D], F32, kind="ExternalInput")
    g1 = nc.dram_tensor("g1", [L, D], F32, kind="ExternalInput")
    c1 = nc.dram_tensor("c1", [L, D], F32, kind="ExternalInput")
    g2 = nc.dram_tensor("g2", [L, D], F32, kind="ExternalInput")
    c2 = nc.dram_tensor("c2", [L, D], F32, kind="ExternalInput")
    out = nc.dram_tensor("out", [D, SB], F32, kind="ExternalOutput")

    agin = [nc.dram_tensor(f"agin{l}", [D, SB], BF16) for l in range(L - 1)]
    agout = [nc.dram_tensor(f"agout{l}", [4 * D, SB], BF16) for l in range(L - 1)]

    with tile.TileContext(nc) as tc, ExitStack() as ctx:
        sing = ctx.enter_context(tc.tile_pool(name="sing", bufs=1))
        biasp = ctx.enter_context(tc.tile_pool(name="biasp", bufs=1))
        hfp = ctx.enter_context(tc.tile_pool(name="hfp", bufs=1))
        hmp = ctx.enter_context(tc.tile_pool(name="hmp", bufs=2))
        kp = ctx.enter_context(tc.tile_pool(name="kp", bufs=1))
        qp = ctx.enter_context(tc.tile_pool(name="qp", bufs=1))
        vtp = ctx.enter_context(tc.tile_pool(name="vtp", bufs=1))
        scrA = ctx.enter_context(tc.tile_pool(name="scrA", bufs=2))
        attp = ctx.enter_context(tc.tile_pool(name="attp", bufs=1))
        x1p = ctx.enter_context(tc.tile_pool(name="x1p", bufs=1))
        vp = ctx.enter_context(tc.tile_pool(name="vp", bufs=1))
        up = ctx.enter_context(tc.tile_pool(name="up", bufs=1))
        posp = ctx.enter_context(tc.tile_pool(name="posp", bufs=1))
        wp = ctx.enter_context(tc.tile_pool(name="wp", bufs=2))
        sqp = ctx.enter_context(tc.tile_pool(name="sqp", bufs=2))
        statp = ctx.enter_context(tc.tile_pool(name="statp", bufs=1))
        rdp = ctx.enter_context(tc.tile_pool(name="rdp", bufs=1))

        # PSUM: 3 pools, statically 4 + 2 + 2 banks = 16KB.
        psBig = ctx.enter_context(tc.tile_pool(name="psBig", bufs=2, space="PSUM"))
        psCtx = ctx.enter_context(tc.tile_pool(name="psCtx", bufs=2, space="PSUM"))
        psBc = ctx.enter_context(tc.tile_pool(name="psBc", bufs=2, space="PSUM"))

        def big_ps(nm):
            return psBig.tile([P, 1024], F32, name=nm, tag="t")

        def ctx_ps(nm):
            return psCtx.tile([P, SB], F32, name=nm, tag="t")

        def bc_ps(nm):
            return psBc.tile([P, SB], F32, name=nm, tag="t")

        ident = sing.tile([P, P], F32)
        make_identity(nc, ident[:])
        ones_col = sing.tile([P, 1], BF16)
        nc.vector.memset(ones_col[:], 1.0)
        ones_row = sing.tile([1, P], F32)
        nc.vector.memset(ones_row[:], 1.0)
        eps_sb = sing.tile([1, 1], F32)
        nc.vector.memset(eps_sb[:], LN_EPS)

        # ---------- embedding ----------
        idx_sb = sing.tile([P, NS], I32)
        nc.sync.dma_start(out=idx_sb[:], in_=idx.rearrange("(c p) -> p c", p=P))
        idxm_sb = sing.tile([P, NSB], I32)
        nc.sync.dma_start(out=idxm_sb[:], in_=idxm.rearrange("(c p) -> p c", p=P))

        hf = hfp.tile([P, ND, S], BF16)       # full-row h, feature-major
        hm = hmp.tile([P, ND, SB], BF16, name="hm", tag="hm")  # my-shard h

        def embed(n_chunks, isb, dst, post):
            for ci in range(n_chunks):
                tok = scrA.tile([P, D], F32, name="tok", tag="a0")
                nc.gpsimd.indirect_dma_start(
                    out=tok[:], out_offset=None, in_=emb[:],
                    in_offset=bass.IndirectOffsetOnAxis(ap=isb[:, ci:ci + 1], axis=0))
                for j in range(ND):
                    ps = bc_ps("etp")
                    nc.tensor.transpose(out=ps[:, 0:P], in_=tok[:, j * P:(j + 1) * P],
                                        identity=ident[:])
                    pt = posp.tile([P, P], F32, name="pt", tag="pt")
                    nc.sync.dma_start(out=pt[:], in_=post[j * P:(j + 1) * P, ci * P:(ci + 1) * P])
                    nc.vector.tensor_add(out=dst[:, j, ci * P:(ci + 1) * P],
                                         in0=ps[:, 0:P], in1=pt[:])

        embed(NS, idx_sb, hf, posT)
        embed(NSB, idxm_sb, hm, posTm)

        BISECT = int(os.environ.get("BISECT_STAGE", "99"))

        def dump_and_done(src_tile, nchunks=ND):
            hn32 = up.tile([P, ND, SB], F32, name="hn32", tag="u")
            for i in range(nchunks):
                nc.vector.tensor_copy(out=hn32[:, i, :], in_=src_tile[:, i, :])
            nc.sync.dma_start(
                out=out[:].rearrange("(j p) c -> p j c", p=P), in_=hn32[:])

        if BISECT == 0:
            dump_and_done(hm)

        # ---------- layers ----------
        for l in range(L if BISECT > 0 else 0):
            wq_t = wp.tile([P, P], BF16, tag="wq")
            nc.sync.dma_start(out=wq_t[:], in_=wqT[l])
            wk_t = wp.tile([P, P], BF16, tag="wk")
            nc.sync.dma_start(out=wk_t[:], in_=wkT[l])
            wv_t = wp.tile([P, DK], BF16, tag="wv")
            nc.sync.dma_start(out=wv_t[0:DK, :], in_=wvT[l])
            nc.sync.dma_start(out=wv_t[DK:P, :], in_=wvT[l])
            bo_sb = biasp.tile([P, ND], F32, tag="bo")
            nc.sync.dma_start(out=bo_sb[:], in_=bo[l].rearrange("(j p) -> p j", p=P))
            b1_sb = biasp.tile([P, NF], F32, tag="b1")
            nc.sync.dma_start(out=b1_sb[:], in_=b1[l].rearrange("(j p) -> p j", p=P))
            b2_sb = biasp.tile([P, ND], F32, tag="b2")
            nc.sync.dma_start(out=b2_sb[:], in_=b2[l].rearrange("(j p) -> p j", p=P))
            g1_sb = biasp.tile([P, ND], F32, tag="g1")
            nc.sync.dma_start(out=g1_sb[:], in_=g1[l].rearrange("(j p) -> p j", p=P))
            c1_sb = biasp.tile([P, ND], F32, tag="c1")
            nc.sync.dma_start(out=c1_sb[:], in_=c1[l].rearrange("(j p) -> p j", p=P))
            g2_sb = biasp.tile([P, ND], F32, tag="g2")
            nc.sync.dma_start(out=g2_sb[:], in_=g2[l].rearrange("(j p) -> p j", p=P))
            c2_sb = biasp.tile([P, ND], F32, tag="c2")
            nc.sync.dma_start(out=c2_sb[:], in_=c2[l].rearrange("(j p) -> p j", p=P))

            # ---- phase A: V^T, K, Q projections ----
            # Even heads (array rows 0-63) write PSUM bank A (cols 0-511),
            # odd heads (rows 64-127) write bank B (cols 512-1023) so the
            # concurrent row-tiled matmuls never share a PSUM bank.
            vt = vtp.tile([P, NS, H, DK + 1], BF16)
            for ci in range(NS):
                psv = big_ps("psv")
                for h in range(H):
                    j, r = h // 2, (h % 2) * DK
                    pos = (h % 2) * 512 + (h // 2) * DK
                    nc.tensor.matmul(
                        psv[:, pos:pos + DK],
                        lhsT=hf[r:r + DK, j, ci * P:(ci + 1) * P],
                        rhs=wv_t[r:r + DK, :], start=True, stop=True)
                nc.vector.tensor_copy(
                    out=vt[:, ci, :, 0:DK].rearrange("p (t o) e -> p o t e", o=2),
                    in_=psv[:].rearrange("p (o t e) -> p o t e", o=2, t=H // 2))
                nc.vector.memset(vt[:, ci, :, DK:DK + 1], 1.0)

            k_sb = kp.tile([P, ND, S], BF16)
            for j in range(ND):
                for t in range(S // 1024):
                    ps = big_ps("kps")
                    for half in range(2):
                        nc.tensor.matmul(
                            ps[:, half * 512:(half + 1) * 512],
                            lhsT=wk_t[:],
                            rhs=hf[:, j, t * 1024 + half * 512:t * 1024 + (half + 1) * 512],
                            start=True, stop=True)
                    nc.vector.tensor_copy(out=k_sb[:, j, t * 1024:(t + 1) * 1024], in_=ps[:])
            q_sb = qp.tile([P, ND, SB], BF16)
            for j in range(ND):
                ps = bc_ps("qps")
                nc.tensor.matmul(ps[:], lhsT=wq_t[:], rhs=hm[:, j, :],
                                 start=True, stop=True)
                nc.vector.tensor_copy(out=q_sb[:, j, :], in_=ps[:])
            if BISECT == 1:
                dump_and_done(q_sb)
                break

            # ---- phase B: attention per head pair ----
            att = attp.tile([P, ND, SB], BF16)
            for j in range(ND):
                ctx0 = ctx_ps("ctx0")
                ctx1 = ctx_ps("ctx1")
                for kc2 in range(NS // 2):
                    kc = 2 * kc2
                    e0 = big_ps("e0")
                    e1 = big_ps("e1")
                    for half in range(2):
                        cc = kc + half
                        nc.tensor.matmul(
                            e0[:, half * 512:(half + 1) * 512],
                            lhsT=k_sb[0:DK, j, cc * P:(cc + 1) * P],
                            rhs=q_sb[0:DK, j, :], start=True, stop=True)
                        nc.tensor.matmul(
                            e1[:, half * 512:(half + 1) * 512],
                            lhsT=k_sb[DK:P, j, cc * P:(cc + 1) * P],
                            rhs=q_sb[DK:P, j, :], start=True, stop=True)
                    a0 = scrA.tile([P, 1024], BF16, name="a0", tag="a0")
                    a1 = scrA.tile([P, 1024], BF16, name="a1", tag="a1")
                    nc.scalar.activation(out=a0[:], in_=e0[:], func=AF.Exp, scale=SCALE)
                    nc.scalar.activation(out=a1[:], in_=e1[:], func=AF.Exp, scale=SCALE)
                    for half in range(2):
                        cc = kc + half
                        nc.tensor.matmul(
                            ctx0[0:DK + 1, :], lhsT=vt[:, cc, 2 * j, :],
                            rhs=a0[:, half * 512:(half + 1) * 512],
                            start=(cc == 0), stop=(cc == NS - 1))
                        nc.tensor.matmul(
                            ctx1[0:DK + 1, :], lhsT=vt[:, cc, 2 * j + 1, :],
                            rhs=a1[:, half * 512:(half + 1) * 512],
                            start=(cc == 0), stop=(cc == NS - 1))
                for hh, cps in ((0, ctx0), (1, ctx1)):
                    rden = rdp.tile([1, SB], F32, name="rden", tag="rden")
                    nc.vector.reciprocal(rden[:], cps[DK:DK + 1, :])
                    rb = bc_ps("rb_att")
                    nc.tensor.matmul(rb[0:DK, :], lhsT=ones_row[:, 0:DK], rhs=rden[:],
                                     start=True, stop=True)
                    rb_sb = rdp.tile([DK, SB], BF16, name="rb_sb", tag="rbsb")
                    nc.vector.tensor_copy(out=rb_sb[:], in_=rb[0:DK, :])
                    nc.vector.tensor_mul(
                        out=att[hh * DK:(hh + 1) * DK, j, :],
                        in0=cps[0:DK, :], in1=rb_sb[:])

            if BISECT == 2:
                dump_and_done(att)
                break

            # ---- phase C: wo projection + residual + LN1 ----
            v_sb = vp.tile([P, ND, SB], BF16, name="v_sb", tag="vres")
            for i in range(ND):
                wo_t = wp.tile([P, ND * P], BF16, tag="w8")
                nc.sync.dma_start(
                    out=wo_t[:].rearrange("p (j c) -> p j c", j=ND),
                    in_=woT[l, :, i * P:(i + 1) * P].rearrange("(j p) c -> p j c", p=P))
                ps = bc_ps("wops")
                for j in range(ND):
                    nc.tensor.matmul(
                        ps[:], lhsT=wo_t[:, j * P:(j + 1) * P],
                        rhs=att[:, j, :], start=(j == 0), stop=(j == ND - 1))
                nc.vector.scalar_tensor_tensor(
                    out=v_sb[:, i, :], in0=ps[:], scalar=bo_sb[:, i:i + 1],
                    in1=hm[:, i, :], op0=ALU.add, op1=ALU.add)

            x1 = x1p.tile([P, ND, SB], BF16)
            _ln(nc, bc_ps, sqp, statp, ones_col, ones_row, eps_sb,
                v_sb, x1, g1_sb, c1_sb)

            if BISECT == 3:
                dump_and_done(x1)
                break

            # ---- phase D: FFN + residual + LN2 ----
            u = up.tile([P, NF, SB], BF16, name="u", tag="u")
            for f in range(NF):
                w1_t = wp.tile([P, ND * P], BF16, tag="w8")
                nc.sync.dma_start(
                    out=w1_t[:].rearrange("p (j c) -> p j c", j=ND),
                    in_=w1T[l, :, f * P:(f + 1) * P].rearrange("(j p) c -> p j c", p=P))
                ups = bc_ps("ups")
                for j in range(ND):
                    nc.tensor.matmul(
                        ups[:], lhsT=w1_t[:, j * P:(j + 1) * P],
                        rhs=x1[:, j, :], start=(j == 0), stop=(j == ND - 1))
                nc.scalar.activation(out=u[:, f, :], in_=ups[:],
                                     func=AF.Relu, bias=b1_sb[:, f:f + 1], scale=1.0)

            v2 = vp.tile([P, ND, SB], BF16, name="v2", tag="vres")
            for dh in range(4):
                yps = [ctx_ps(f"y{i2}") for i2 in range(2)]
                for f in range(NF):
                    w2_t = wp.tile([P, 2 * P], BF16, tag="w2")
                    nc.sync.dma_start(
                        out=w2_t[:],
                        in_=w2T[l, f * P:(f + 1) * P, dh * 256:(dh + 1) * 256])
                    for i2 in range(2):
                        nc.tensor.matmul(
                            yps[i2][:], lhsT=w2_t[:, i2 * P:(i2 + 1) * P],
                            rhs=u[:, f, :], start=(f == 0), stop=(f == NF - 1))
                for i2 in range(2):
                    i = dh * 2 + i2
                    nc.vector.scalar_tensor_tensor(
                        out=v2[:, i, :], in0=yps[i2][:], scalar=b2_sb[:, i:i + 1],
                        in1=x1[:, i, :], op0=ALU.add, op1=ALU.add)

            last = l == L - 1
            if last:
                hn32 = up.tile([P, ND, SB], F32, name="hn32", tag="u")
                _ln(nc, bc_ps, sqp, statp, ones_col, ones_row, eps_sb,
                    v2, None, g2_sb, c2_sb, F32out=hn32)
                nc.sync.dma_start(
                    out=out[:].rearrange("(j p) c -> p j c", p=P), in_=hn32[:])
            else:
                hn = hmp.tile([P, ND, SB], BF16, name="hn", tag="hm")
                _ln(nc, bc_ps, sqp, statp, ones_col, ones_row, eps_sb,
                    v2, hn, g2_sb, c2_sb)
                if use_ag:
                    nc.sync.dma_start(
                        out=agin[l][:].rearrange("(j p) c -> p j c", p=P), in_=hn[:])
                    nc.gpsimd.collective_compute(
                        "AllGather", ALU.bypass, replica_groups=GROUPS,
                        ins=[agin[l][:]], outs=[agout[l][:]])
                    hf = hfp.tile([P, ND, S], BF16, name="hf")
                    for rr in range(4):
                        nc.sync.dma_start(
                            out=hf[:, :, rr * SB:(rr + 1) * SB],
                            in_=agout[l][rr * D:(rr + 1) * D, :].rearrange(
                                "(j p) c -> p j c", p=P))
                else:
                    hf = hfp.tile([P, ND, S], BF16, name="hf")
                    for rr in range(4):
                        nc.vector.tensor_copy(
                            out=hf[:, :, rr * SB:(rr + 1) * SB], in_=hn[:])
                hm = hn

    nc.compile()
    return nc


def _ln(nc, bc_ps, sqp, statp, ones_col, ones_row, eps_sb, v_sb, x_out,
        g_sb, c_sb, F32out=None):
    """LayerNorm over the partition (feature) axis of feature-major v_sb."""
    nd, sb = ND, SB
    mu_ps = bc_ps("mu")
    sq_ps = bc_ps("sq")
    for i in range(nd):
        nc.tensor.matmul(mu_ps[0:1, :], lhsT=ones_col[:], rhs=v_sb[:, i, :],
                         start=(i == 0), stop=(i == nd - 1))
    for i in range(nd):
        sq = sqp.tile([P, sb], BF16, name="sq", tag="sq")
        nc.scalar.activation(out=sq[:], in_=v_sb[:, i, :], func=AF.Square)
        nc.tensor.matmul(sq_ps[0:1, :], lhsT=ones_col[:], rhs=sq[:],
                         start=(i == 0), stop=(i == nd - 1))
    m_sb = statp.tile([1, sb], F32, name="m", tag="m")
    nc.scalar.mul(m_sb[:], mu_ps[0:1, :], 1.0 / (nd * P))
    var_sb = statp.tile([1, sb], F32, name="var", tag="var")
    nc.scalar.mul(var_sb[:], sq_ps[0:1, :], 1.0 / (nd * P))
    t_sb = statp.tile([1, sb], F32, name="t", tag="t")
    nc.vector.tensor_mul(out=t_sb[:], in0=m_sb[:], in1=m_sb[:])
    nc.vector.tensor_sub(out=var_sb[:], in0=var_sb[:], in1=t_sb[:])
    nc.scalar.activation(out=t_sb[:], in_=var_sb[:], func=AF.Sqrt, bias=eps_sb[:])
    rstd_sb = statp.tile([1, sb], F32, name="rstd", tag="var")
    nc.vector.reciprocal(rstd_sb[:], t_sb[:])

    mb = bc_ps("mb")
    nc.tensor.matmul(mb[:], lhsT=ones_row[:], rhs=m_sb[:], start=True, stop=True)
    rb = bc_ps("rb")
    nc.tensor.matmul(rb[:], lhsT=ones_row[:], rhs=rstd_sb[:], start=True, stop=True)

    for i in range(nd):
        dst = F32out[:, i, :] if F32out is not None else x_out[:, i, :]
        nc.vector.tensor_sub(out=dst, in0=v_sb[:, i, :], in1=mb[:])
        nc.vector.tensor_mul(out=dst, in0=dst, in1=rb[:])
        nc.scalar.activation(out=dst, in_=dst, func=AF.Identity,
                             bias=c_sb[:, i:i + 1], scale=g_sb[:, i:i + 1])


_NC_CACHE = {}


def _get_nc(n_layers=NLAYERS, n_cores=NCORES):
    key = (n_layers, n_cores)
    if key not in _NC_CACHE:
        _NC_CACHE[key] = build_nc(n_layers, n_cores)
    return _NC_CACHE[key]


def prep_in_maps(inputs, n_layers=NLAYERS):
    bf = ml_dtypes.bfloat16
    L = n_layers
    x = np.asarray(inputs["x"]).astype(np.int32)
    emb = np.ascontiguousarray(np.asarray(inputs["emb"], dtype=np.float32))
    pos = np.asarray(inputs["pos"], dtype=np.float32)
    posT = np.ascontiguousarray(pos[:S].T)

    def blk(w):  # [L, 64, 64] -> [L, 128, 128] blockdiag of w.T
        wT = np.transpose(np.asarray(w, dtype=np.float32), (0, 2, 1))[:L]
        o = np.zeros((L, P, P), np.float32)
        o[:, :DK, :DK] = wT
        o[:, DK:, DK:] = wT
        return np.ascontiguousarray(o.astype(bf))

    shared = {
        "emb": emb,
        "posT": posT,
        "wqT": blk(inputs["wq"]),
        "wkT": blk(inputs["wk"]),
        "wvT": np.ascontiguousarray(
            np.transpose(np.asarray(inputs["wv"], dtype=np.float32), (0, 2, 1))[:L].astype(bf)),
        "woT": np.ascontiguousarray(
            np.transpose(np.asarray(inputs["wo"], dtype=np.float32), (0, 2, 1))[:L].astype(bf)),
        "w1T": np.ascontiguousarray(
            np.transpose(np.asarray(inputs["w1"], dtype=np.float32), (0, 2, 1))[:L].astype(bf)),
        "w2T": np.ascontiguousarray(
            np.transpose(np.asarray(inputs["w2"], dtype=np.float32), (0, 2, 1))[:L].astype(bf)),
        "bo": np.ascontiguousarray(np.asarray(inputs["bo"], np.float32)[:L]),
        "b1": np.ascontiguousarray(np.asarray(inputs["b1"], np.float32)[:L]),
        "b2": np.ascontiguousarray(np.asarray(inputs["b2"], np.float32)[:L]),
        "g1": np.ascontiguousarray(np.asarray(inputs["ln1_g"], np.float32)[:L]),
        "c1": np.ascontiguousarray(np.asarray(inputs["ln1_b"], np.float32)[:L]),
        "g2": np.ascontiguousarray(np.asarray(inputs["ln2_g"], np.float32)[:L]),
        "c2": np.ascontiguousarray(np.asarray(inputs["ln2_b"], np.float32)[:L]),
    }
    in_maps = []
    for c in range(NCORES):
        row, b = c // 4, c % 4
        m = dict(shared)
        m["idx"] = np.ascontiguousarray(x[row])
        m["idxm"] = np.ascontiguousarray(x[row, b * SB:(b + 1) * SB])
        m["posTm"] = np.ascontiguousarray(posT[:, b * SB:(b + 1) * SB])
        in_maps.append(m)
    return in_maps


def run(inputs, n_layers=NLAYERS, trace=False):
    nc = _get_nc(n_layers, NCORES)
    in_maps = prep_in_maps(inputs, n_layers)
    res = run_bass_kernel_spmd(nc, in_maps, core_ids=list(range(NCORES)), trace=trace)
    full = np.zeros((B, S, D), np.float32)
    for c in range(NCORES):
        row, b = c // 4, c % 4
        full[row, b * SB:(b + 1) * SB, :] = res.results[c]["out"].T
    return full, res


def run_timed(inputs, n_layers=NLAYERS, iters=6):
    """Time the compiled NEFF with device-resident inputs (min over iters)."""
    import time
    import jax
    from jax.sharding import Mesh, PartitionSpec, NamedSharding
    from jax.experimental.shard_map import shard_map
    from concourse import mybir as _mybir
    from concourse.bass2jax import _bass_exec_p, install_neuronx_cc_hook, partition_id_tensor

    nc = _get_nc(n_layers, NCORES)
    in_maps = prep_in_maps(inputs, n_layers)
    install_neuronx_cc_hook()

    partition_name = nc.partition_id_tensor.name if nc.partition_id_tensor else None
    in_names, out_names, out_avals, zero_outs = [], [], [], []
    for alloc in nc.m.functions[0].allocations:
        if not isinstance(alloc, _mybir.MemoryLocationSet):
            continue
        name = alloc.memorylocations[0].name
        if alloc.kind == "ExternalInput":
            if name != partition_name:
                in_names.append(name)
        elif alloc.kind == "ExternalOutput":
            shape = tuple(alloc.tensor_shape)
            dtype = _mybir.dt.np(alloc.dtype)
            out_names.append(name)
            out_avals.append(jax.core.ShapedArray(shape, dtype))
            zero_outs.append(np.zeros(shape, dtype))
    n_params = len(in_names)
    n_outs = len(out_names)
    all_in_names = list(in_names) + list(out_names)
    if partition_name is not None:
        all_in_names.append(partition_name)

    def _body(*args):
        operands = list(args)
        if partition_name is not None:
            operands.append(partition_id_tensor())
        return tuple(_bass_exec_p.bind(
            *operands, out_avals=tuple(out_avals), in_names=tuple(all_in_names),
            out_names=tuple(out_names), lowering_input_output_aliases=(),
            sim_require_finite=True, sim_require_nnan=True, nc=nc))

    devices = jax.devices()[:NCORES]
    mesh = Mesh(np.asarray(devices), ("core",))
    nshard = NamedSharding(mesh, PartitionSpec("core"))
    donate = tuple(range(n_params, n_params + n_outs))
    fn = jax.jit(shard_map(_body, mesh=mesh,
                           in_specs=(PartitionSpec("core"),) * (n_params + n_outs),
                           out_specs=(PartitionSpec("core"),) * n_outs,
                           check_rep=False), donate_argnums=donate, keep_unused=True)
    concat_in = [np.concatenate([np.asarray(in_maps[c][nm]) for c in range(NCORES)], axis=0)
                 for nm in in_names]
    concat_zeros = [np.zeros((NCORES * z.shape[0], *z.shape[1:]), z.dtype) for z in zero_outs]
    dev_in = [jax.device_put(a, nshard) for a in concat_in]
    jax.block_until_ready(dev_in)

    def one_call():
        dz = [jax.device_put(z, nshard) for z in concat_zeros]
        jax.block_until_ready(dz)
        t0 = time.perf_counter()
        outs = fn(*dev_in, *dz)
        jax.block_until_ready(outs)
        return time.perf_counter() - t0, outs

    _, outs = one_call()  # compile + warm
    times = []
    for _ in range(iters):
        dt, outs = one_call()
        times.append(dt)
    full = np.zeros((B, S, D), np.float32)
    arr = np.asarray(outs[out_names.index("out")]).reshape(NCORES, D, SB)
    for c in range(NCORES):
        row, b = c // 4, c % 4
        full[row, b * SB:(b + 1) * SB, :] = arr[c].T
    return full, min(times), times


def kernel(**inputs):
    full, _ = run(inputs)
    return full


# revision 21
# speedup vs baseline: 17.4085x; 17.4085x over previous
"""Trainium2 Bass kernel for a 6-layer dense transformer encoder.

Sharding: 8 cores = 2 batch rows x 4 token-blocks (512 tokens each).
Activations are kept feature-major ([D partitions, S free]).  Each core
holds its full 2048-token row of h in bf16 (for K/V projections) plus its
own 512-token shard (for Q / residual stream).  A 4-way AllGather per
layer rebuilds the full row from the per-core layer outputs.

Compute: bf16 matmuls with f32 PSUM accumulation; LayerNorm statistics
via ones-vector matmuls on the tensor engine (partition-axis reductions);
softmax without max-subtraction (logits provably tiny); softmax
denominator folded into the A@V matmul through a ones-column appended to
the transposed V tiles.
"""

import os
import numpy as np
import ml_dtypes
from contextlib import ExitStack

import concourse.bass as bass
import concourse.bacc as bacc
import concourse.tile as tile
import concourse.mybir as mybir
from concourse.bass_utils import run_bass_kernel_spmd
from concourse.masks import make_identity

F32 = mybir.dt.float32
BF16 = mybir.dt.bfloat16
I32 = mybir.dt.int32
AF = mybir.ActivationFunctionType
ALU = mybir.AluOpType

VOCAB, MAXLEN, NLAYERS, D, H, DK, DFF = 32000, 2048, 6, 1024, 16, 64, 4096
B, S = 2, 2048
P = 128
ND = D // P          # 8 d-chunks
NF = DFF // P        # 32 f-chunks
NS = S // P          # 16 s-chunks per row
SB = 512             # tokens per core
NSB = SB // P        # 4 s-chunks per core shard
NCORES = 8
GROUPS = [[0, 1, 2, 3], [4, 5, 6, 7]]
LN_EPS = 1e-5
SCALE = 1.0 / 8.0    # 1/sqrt(DK)


def build_nc(n_layers=NLAYERS, n_cores=NCORES):
    nc = bacc.Bacc(None, target_bir_lowering=False, num_devices=n_cores)
    L = n_layers
    use_ag = n_cores > 1

    emb = nc.dram_tensor("emb", [VOCAB, D], F32, kind="ExternalInput")
    idx = nc.dram_tensor("idx", [S], I32, kind="ExternalInput")
    idxm = nc.dram_tensor("idxm", [SB], I32, kind="ExternalInput")
    posT = nc.dram_tensor("posT", [D, S], F32, kind="ExternalInput")
    posTm = nc.dram_tensor("posTm", [D, SB], F32, kind="ExternalInput")
    wqT = nc.dram_tensor("wqT", [L, P, P], BF16, kind="ExternalInput")
    wkT = nc.dram_tensor("wkT", [L, P, P], BF16, kind="ExternalInput")
    wvT = nc.dram_tensor("wvT", [L, DK, DK], BF16, kind="ExternalInput")
    woT = nc.dram_tensor("woT", [L, D, D], BF16, kind="ExternalInput")
    w1T = nc.dram_tensor("w1T", [L, D, DFF], BF16, kind="ExternalInput")
    w2T = nc.dram_tensor("w2T", [L, DFF, D], BF16, kind="ExternalInput")
    bo = nc.dram_tensor("bo", [L, D], F32, kind="ExternalInput")
    b1 = nc.dram_tensor("b1", [L, DFF], F32, kind="ExternalInput")
    b2 = nc.dram_tensor("b2", [L, D], F32, kind="ExternalInput")
    g1 = nc.dram_tensor("g1", [L, D], F32, kind="ExternalInput")
    c1 = nc.dram_tensor("c1", [L, D], F32, kind="ExternalInput")
    g2 = nc.dram_tensor("g2", [L, D], F32, kind="ExternalInput")
    c2 = nc.dram_tensor("c2", [L, D], F32, kind="ExternalInput")
    out = nc.dram_tensor("out", [D, SB], F32, kind="ExternalOutput")

    agin = [nc.dram_tensor(f"agin{l}", [D, SB], BF16) for l in range(L - 1)]
    agout = [nc.dram_tensor(f"agout{l}", [4 * D, SB], BF16) for l in range(L - 1)]

    with tile.TileContext(nc) as tc, ExitStack() as ctx:
        sing = ctx.enter_context(tc.tile_pool(name="sing", bufs=1))
        biasp = ctx.enter_context(tc.tile_pool(name="biasp", bufs=1))
        hfp = ctx.enter_context(tc.tile_pool(name="hfp", bufs=1))
        hmp = ctx.enter_context(tc.tile_pool(name="hmp", bufs=2))
        kp = ctx.enter_context(tc.tile_pool(name="kp", bufs=1))
        qp = ctx.enter_context(tc.tile_pool(name="qp", bufs=1))
        vtp = ctx.enter_context(tc.tile_pool(name="vtp", bufs=1))
        scrA = ctx.enter_context(tc.tile_pool(name="scrA", bufs=2))
        attp = ctx.enter_context(tc.tile_pool(name="attp", bufs=1))
        x1p = ctx.enter_context(tc.tile_pool(name="x1p", bufs=1))
        vp = ctx.enter_context(tc.tile_pool(name="vp", bufs=1))
        up = ctx.enter_context(tc.tile_pool(name="up", bufs=1))
        posp = ctx.enter_context(tc.tile_pool(name="posp", bufs=1))
        wp = ctx.enter_context(tc.tile_pool(name="wp", bufs=2))
        sqp = ctx.enter_context(tc.tile_pool(name="sqp", bufs=2))
        statp = ctx.enter_context(tc.tile_pool(name="statp", bufs=1))
        rdp = ctx.enter_context(tc.tile_pool(name="rdp", bufs=1))

        # PSUM: 3 pools, statically 4 + 2 + 2 banks = 16KB.
        psBig = ctx.enter_context(tc.tile_pool(name="psBig", bufs=2, space="PSUM"))
        psCtx = ctx.enter_context(tc.tile_pool(name="psCtx", bufs=2, space="PSUM"))
        psBc = ctx.enter_context(tc.tile_pool(name="psBc", bufs=2, space="PSUM"))

        def big_ps(nm):
            return psBig.tile([P, 1024], F32, name=nm, tag="t")

        def ctx_ps(nm):
            return psCtx.tile([P, SB], F32, name=nm, tag="t")

        def bc_ps(nm):
            return psBc.tile([P, SB], F32, name=nm, tag="t")

        ident = sing.tile([P, P], F32)
        make_identity(nc, ident[:])
        ones_col = sing.tile([P, 1], BF16)
        nc.vector.memset(ones_col[:], 1.0)
        ones_row = sing.tile([1, P], F32)
        nc.vector.memset(ones_row[:], 1.0)
        eps_sb = sing.tile([1, 1], F32)
        nc.vector.memset(eps_sb[:], LN_EPS)

        # ---------- embedding ----------
        idx_sb = sing.tile([P, NS], I32)
        nc.sync.dma_start(out=idx_sb[:], in_=idx.rearrange("(c p) -> p c", p=P))
        idxm_sb = sing.tile([P, NSB], I32)
        nc.sync.dma_start(out=idxm_sb[:], in_=idxm.rearrange("(c p) -> p c", p=P))

        hf = hfp.tile([P, ND, S], BF16)       # full-row h, feature-major
        hm = hmp.tile([P, ND, SB], BF16, name="hm", tag="hm")  # my-shard h

        def embed(n_chunks, isb, dst, post):
            for ci in range(n_chunks):
                tok = scrA.tile([P, D], F32, name="tok", tag="a0")
                nc.gpsimd.indirect_dma_start(
                    out=tok[:], out_offset=None, in_=emb[:],
                    in_offset=bass.IndirectOffsetOnAxis(ap=isb[:, ci:ci + 1], axis=0))
                for j in range(ND):
                    ps = bc_ps("etp")
                    nc.tensor.transpose(out=ps[:, 0:P], in_=tok[:, j * P:(j + 1) * P],
                                        identity=ident[:])
                    pt = posp.tile([P, P], F32, name="pt", tag="pt")
                    nc.sync.dma_start(out=pt[:], in_=post[j * P:(j + 1) * P, ci * P:(ci + 1) * P])
                    nc.vector.tensor_add(out=dst[:, j, ci * P:(ci + 1) * P],
                                         in0=ps[:, 0:P], in1=pt[:])

        embed(NS, idx_sb, hf, posT)
        embed(NSB, idxm_sb, hm, posTm)

        BISECT = int(os.environ.get("BISECT_STAGE", "99"))

        def dump_and_done(src_tile, nchunks=ND):
            hn32 = up.tile([P, ND, SB], F32, name="hn32", tag="u")
            for i in range(nchunks):
                nc.vector.tensor_copy(out=hn32[:, i, :], in_=src_tile[:, i, :])
            nc.sync.dma_start(
                out=out[:].rearrange("(j p) c -> p j c", p=P), in_=hn32[:])

        if BISECT == 0:
            dump_and_done(hm)

        # ---------- layers ----------
        for l in range(L if BISECT > 0 else 0):
            wq_t = wp.tile([P, P], BF16, tag="wq")
            nc.sync.dma_start(out=wq_t[:], in_=wqT[l])
            wk_t = wp.tile([P, P], BF16, tag="wk")
            nc.sync.dma_start(out=wk_t[:], in_=wkT[l])
            wv_t = wp.tile([P, DK], BF16, tag="wv")
            nc.sync.dma_start(out=wv_t[0:DK, :], in_=wvT[l])
            nc.sync.dma_start(out=wv_t[DK:P, :], in_=wvT[l])
            bo_sb = biasp.tile([P, ND], F32, tag="bo")
            nc.sync.dma_start(out=bo_sb[:], in_=bo[l].rearrange("(j p) -> p j", p=P))
            b1_sb = biasp.tile([P, NF], F32, tag="b1")
            nc.sync.dma_start(out=b1_sb[:], in_=b1[l].rearrange("(j p) -> p j", p=P))
            b2_sb = biasp.tile([P, ND], F32, tag="b2")
            nc.sync.dma_start(out=b2_sb[:], in_=b2[l].rearrange("(j p) -> p j", p=P))
            g1_sb = biasp.tile([P, ND], F32, tag="g1")
            nc.sync.dma_start(out=g1_sb[:], in_=g1[l].rearrange("(j p) -> p j", p=P))
            c1_sb = biasp.tile([P, ND], F32, tag="c1")
            nc.sync.dma_start(out=c1_sb[:], in_=c1[l].rearrange("(j p) -> p j", p=P))
            g2_sb = biasp.tile([P, ND], F32, tag="g2")
            nc.sync.dma_start(out=g2_sb[:], in_=g2[l].rearrange("(j p) -> p j", p=P))
            c2_sb = biasp.tile([P, ND], F32, tag="c2")
            nc.sync.dma_start(out=c2_sb[:], in_=c2[l].rearrange("(j p) -> p j", p=P))

            # ---- phase A: V^T, K, Q projections ----
            # Even heads (array rows 0-63) write PSUM bank A (cols 0-511),
            # odd heads (rows 64-127) write bank B (cols 512-1023) so the
            # concurrent row-tiled matmuls never share a PSUM bank.
            vt = vtp.tile([P, NS, H, DK + 1], BF16)
            for ci in range(NS):
                psv = big_ps("psv")
                for h in range(H):
                    j, r = h // 2, (h % 2) * DK
                    pos = (h % 2) * 512 + (h // 2) * DK
                    nc.tensor.matmul(
                        psv[:, pos:pos + DK],
                        lhsT=hf[r:r + DK, j, ci * P:(ci + 1) * P],
                        rhs=wv_t[r:r + DK, :], start=True, stop=True)
                nc.vector.tensor_copy(
                    out=vt[:, ci, :, 0:DK].rearrange("p (t o) e -> p o t e", o=2),
                    in_=psv[:].rearrange("p (o t e) -> p o t e", o=2, t=H // 2))
                nc.vector.memset(vt[:, ci, :, DK:DK + 1], 1.0)

            k_sb = kp.tile([P, ND, S], BF16)
            for j in range(ND):
                for t in range(S // 1024):
                    ps = big_ps("kps")
                    for half in range(2):
                        nc.tensor.matmul(
                            ps[:, half * 512:(half + 1) * 512],
                            lhsT=wk_t[:],
                            rhs=hf[:, j, t * 1024 + half * 512:t * 1024 + (half + 1) * 512],
                            start=True, stop=True)
                    nc.vector.tensor_copy(out=k_sb[:, j, t * 1024:(t + 1) * 1024], in_=ps[:])
            q_sb = qp.tile([P, ND, SB], BF16)
            for j in range(ND):
                ps = bc_ps("qps")
                nc.tensor.matmul(ps[:], lhsT=wq_t[:], rhs=hm[:, j, :],
                                 start=True, stop=True)
                nc.vector.tensor_copy(out=q_sb[:, j, :], in_=ps[:])
            if BISECT == 1:
                dump_and_done(q_sb)
                break

            # ---- phase B: attention per head pair ----
            att = attp.tile([P, ND, SB], BF16)
            for j in range(ND):
                ctx0 = ctx_ps("ctx0")
                ctx1 = ctx_ps("ctx1")
                for kc2 in range(NS // 2):
                    kc = 2 * kc2
                    e0 = big_ps("e0")
                    e1 = big_ps("e1")
                    for half in range(2):
                        cc = kc + half
                        nc.tensor.matmul(
                            e0[:, half * 512:(half + 1) * 512],
                            lhsT=k_sb[0:DK, j, cc * P:(cc + 1) * P],
                            rhs=q_sb[0:DK, j, :], start=True, stop=True)
                        nc.tensor.matmul(
                            e1[:, half * 512:(half + 1) * 512],
                            lhsT=k_sb[DK:P, j, cc * P:(cc + 1) * P],
                            rhs=q_sb[DK:P, j, :], start=True, stop=True)
                    a0 = scrA.tile([P, 1024], BF16, name="a0", tag="a0")
                    a1 = scrA.tile([P, 1024], BF16, name="a1", tag="a1")
                    nc.scalar.activation(out=a0[:], in_=e0[:], func=AF.Exp, scale=SCALE)
                    nc.scalar.activation(out=a1[:], in_=e1[:], func=AF.Exp, scale=SCALE)
                    for half in range(2):
                        cc = kc + half
                        nc.tensor.matmul(
                            ctx0[0:DK + 1, :], lhsT=vt[:, cc, 2 * j, :],
                            rhs=a0[:, half * 512:(half + 1) * 512],
                            start=(cc == 0), stop=(cc == NS - 1))
                        nc.tensor.matmul(
                            ctx1[0:DK + 1, :], lhsT=vt[:, cc, 2 * j + 1, :],
                            rhs=a1[:, half * 512:(half + 1) * 512],
                            start=(cc == 0), stop=(cc == NS - 1))
                for hh, cps in ((0, ctx0), (1, ctx1)):
                    rden = rdp.tile([1, SB], F32, name="rden", tag="rden")
                    nc.vector.reciprocal(rden[:], cps[DK:DK + 1, :])
                    rb = bc_ps("rb_att")
                    nc.tensor.matmul(rb[0:DK, :], lhsT=ones_row[:, 0:DK], rhs=rden[:],
                                     start=True, stop=True)
                    rb_sb = rdp.tile([DK, SB], BF16, name="rb_sb", tag="rbsb")
                    nc.vector.tensor_copy(out=rb_sb[:], in_=rb[0:DK, :])
                    nc.vector.tensor_mul(
                        out=att[hh * DK:(hh + 1) * DK, j, :],
                        in0=cps[0:DK, :], in1=rb_sb[:])

            if BISECT == 2:
                dump_and_done(att)
                break

            # ---- phase C: wo projection + residual + LN1 ----
            v_sb = vp.tile([P, ND, SB], BF16, name="v_sb", tag="vres")
            for i in range(ND):
                wo_t = wp.tile([P, ND * P], BF16, tag="w8")
                nc.sync.dma_start(
                    out=wo_t[:].rearrange("p (j c) -> p j c", j=ND),
                    in_=woT[l, :, i * P:(i + 1) * P].rearrange("(j p) c -> p j c", p=P))
                ps = bc_ps("wops")
                for j in range(ND):
                    nc.tensor.matmul(
                        ps[:], lhsT=wo_t[:, j * P:(j + 1) * P],
                        rhs=att[:, j, :], start=(j == 0), stop=(j == ND - 1))
                nc.vector.scalar_tensor_tensor(
                    out=v_sb[:, i, :], in0=ps[:], scalar=bo_sb[:, i:i + 1],
                    in1=hm[:, i, :], op0=ALU.add, op1=ALU.add)

            x1 = x1p.tile([P, ND, SB], BF16)
            _ln(nc, bc_ps, sqp, statp, ones_col, ones_row, eps_sb,
                v_sb, x1, g1_sb, c1_sb)

            if BISECT == 3:
                dump_and_done(x1)
                break

            # ---- phase D: FFN + residual + LN2 ----
            u = up.tile([P, NF, SB], BF16, name="u", tag="u")
            for f in range(NF):
                w1_t = wp.tile([P, ND * P], BF16, tag="w8")
                nc.sync.dma_start(
                    out=w1_t[:].rearrange("p (j c) -> p j c", j=ND),
                    in_=w1T[l, :, f * P:(f + 1) * P].rearrange("(j p) c -> p j c", p=P))
                ups = bc_ps("ups")
                for j in range(ND):
                    nc.tensor.matmul(
                        ups[:], lhsT=w1_t[:, j * P:(j + 1) * P],
                        rhs=x1[:, j, :], start=(j == 0), stop=(j == ND - 1))
                nc.scalar.activation(out=u[:, f, :], in_=ups[:],
                                     func=AF.Relu, bias=b1_sb[:, f:f + 1], scale=1.0)

            v2 = vp.tile([P, ND, SB], BF16, name="v2", tag="vres")
            for dh in range(4):
                yps = [ctx_ps(f"y{i2}") for i2 in range(2)]
                for f in range(NF):
                    w2_t = wp.tile([P, 2 * P], BF16, tag="w2")
                    nc.sync.dma_start(
                        out=w2_t[:],
                        in_=w2T[l, f * P:(f + 1) * P, dh * 256:(dh + 1) * 256])
                    for i2 in range(2):
                        nc.tensor.matmul(
                            yps[i2][:], lhsT=w2_t[:, i2 * P:(i2 + 1) * P],
                            rhs=u[:, f, :], start=(f == 0), stop=(f == NF - 1))
                for i2 in range(2):
                    i = dh * 2 + i2
                    nc.vector.scalar_tensor_tensor(
                        out=v2[:, i, :], in0=yps[i2][:], scalar=b2_sb[:, i:i + 1],
                        in1=x1[:, i, :], op0=ALU.add, op1=ALU.add)

            last = l == L - 1
            if last:
                hn32 = up.tile([P, ND, SB], F32, name="hn32", tag="u")
                _ln(nc, bc_ps, sqp, statp, ones_col, ones_row, eps_sb,
                    v2, None, g2_sb, c2_sb, F32out=hn32)
                nc.sync.dma_start(
                    out=out[:].rearrange("(j p) c -> p j c", p=P), in_=hn32[:])
            else:
                hn = hmp.tile([P, ND, SB], BF16, name="hn", tag="hm")
                _ln(nc, bc_ps, sqp, statp, ones_col, ones_row, eps_sb,
                    v2, hn, g2_sb, c2_sb)
                if use_ag:
                    nc.sync.dma_start(
                        out=agin[l][:].rearrange("(j p) c -> p j c", p=P), in_=hn[:])
                    nc.gpsimd.collective_compute(
                        "AllGather", ALU.bypass, replica_groups=GROUPS,
                        ins=[agin[l][:]], outs=[agout[l][:]])
                    hf = hfp.tile([P, ND, S], BF16, name="hf")
                    for rr in range(4):
                        nc.sync.dma_start(
                            out=hf[:, :, rr * SB:(rr + 1) * SB],
                            in_=agout[l][rr * D:(rr + 1) * D, :].rearrange(
                                "(j p) c -> p j c", p=P))
                else:
                    hf = hfp.tile([P, ND, S], BF16, name="hf")
                    for rr in range(4):
                        nc.vector.tensor_copy(
                            out=hf[:, :, rr * SB:(rr + 1) * SB], in_=hn[:])
                hm = hn

    nc.compile()
    return nc


def _ln(nc, bc_ps, sqp, statp, ones_col, ones_row, eps_sb, v_sb, x_out,
        g_sb, c_sb, F32out=None):
    """LayerNorm over the partition (feature) axis of feature-major v_sb."""
    nd, sb = ND, SB
    mu_ps = bc_ps("mu")
    sq_ps = bc_ps("sq")
    for i in range(nd):
        nc.tensor.matmul(mu_ps[0:1, :], lhsT=ones_col[:], rhs=v_sb[:, i, :],
                         start=(i == 0), stop=(i == nd - 1))
    for i in range(nd):
        sq = sqp.tile([P, sb], BF16, name="sq", tag="sq")
        nc.scalar.activation(out=sq[:], in_=v_sb[:, i, :], func=AF.Square)
        nc.tensor.matmul(sq_ps[0:1, :], lhsT=ones_col[:], rhs=sq[:],
                         start=(i == 0), stop=(i == nd - 1))
    m_sb = statp.tile([1, sb], F32, name="m", tag="m")
    nc.scalar.mul(m_sb[:], mu_ps[0:1, :], 1.0 / (nd * P))
    var_sb = statp.tile([1, sb], F32, name="var", tag="var")
    nc.scalar.mul(var_sb[:], sq_ps[0:1, :], 1.0 / (nd * P))
    t_sb = statp.tile([1, sb], F32, name="t", tag="t")
    nc.vector.tensor_mul(out=t_sb[:], in0=m_sb[:], in1=m_sb[:])
    nc.vector.tensor_sub(out=var_sb[:], in0=var_sb[:], in1=t_sb[:])
    nc.scalar.activation(out=t_sb[:], in_=var_sb[:], func=AF.Sqrt, bias=eps_sb[:])
    rstd_sb = statp.tile([1, sb], F32, name="rstd", tag="var")
    nc.vector.reciprocal(rstd_sb[:], t_sb[:])

    mb = bc_ps("mb")
    nc.tensor.matmul(mb[:], lhsT=ones_row[:], rhs=m_sb[:], start=True, stop=True)
    rb = bc_ps("rb")
    nc.tensor.matmul(rb[:], lhsT=ones_row[:], rhs=rstd_sb[:], start=True, stop=True)

    for i in range(nd):
        dst = F32out[:, i, :] if F32out is not None else x_out[:, i, :]
        nc.vector.tensor_sub(out=dst, in0=v_sb[:, i, :], in1=mb[:])
        nc.vector.tensor_mul(out=dst, in0=dst, in1=rb[:])
        nc.scalar.activation(out=dst, in_=dst, func=AF.Identity,
                             bias=c_sb[:, i:i + 1], scale=g_sb[:, i:i + 1])


_NC_CACHE = {}


def _get_nc(n_layers=NLAYERS, n_cores=NCORES):
    key = (n_layers, n_cores)
    if key not in _NC_CACHE:
        _NC_CACHE[key] = build_nc(n_layers, n_cores)
    return _NC_CACHE[key]


def prep_in_maps(inputs, n_layers=NLAYERS):
    bf = ml_dtypes.bfloat16
    L = n_layers
    x = np.asarray(inputs["x"]).astype(np.int32)
    emb = np.ascontiguousarray(np.asarray(inputs["emb"], dtype=np.float32))
    pos = np.asarray(inputs["pos"], dtype=np.float32)
    posT = np.ascontiguousarray(pos[:S].T)

    def blk(w):  # [L, 64, 64] -> [L, 128, 128] blockdiag of w.T
        wT = np.transpose(np.asarray(w, dtype=np.float32), (0, 2, 1))[:L]
        o = np.zeros((L, P, P), np.float32)
        o[:, :DK, :DK] = wT
        o[:, DK:, DK:] = wT
        return np.ascontiguousarray(o.astype(bf))

    shared = {
        "emb": emb,
        "posT": posT,
        "wqT": blk(inputs["wq"]),
        "wkT": blk(inputs["wk"]),
        "wvT": np.ascontiguousarray(
            np.transpose(np.asarray(inputs["wv"], dtype=np.float32), (0, 2, 1))[:L].astype(bf)),
        "woT": np.ascontiguousarray(
            np.transpose(np.asarray(inputs["wo"], dtype=np.float32), (0, 2, 1))[:L].astype(bf)),
        "w1T": np.ascontiguousarray(
            np.transpose(np.asarray(inputs["w1"], dtype=np.float32), (0, 2, 1))[:L].astype(bf)),
        "w2T": np.ascontiguousarray(
            np.transpose(np.asarray(inputs["w2"], dtype=np.float32), (0, 2, 1))[:L].astype(bf)),
        "bo": np.ascontiguousarray(np.asarray(inputs["bo"], np.float32)[:L]),
        "b1": np.ascontiguousarray(np.asarray(inputs["b1"], np.float32)[:L]),
        "b2": np.ascontiguousarray(np.asarray(inputs["b2"], np.float32)[:L]),
        "g1": np.ascontiguousarray(np.asarray(inputs["ln1_g"], np.float32)[:L]),
        "c1": np.ascontiguousarray(np.asarray(inputs["ln1_b"], np.float32)[:L]),
        "g2": np.ascontiguousarray(np.asarray(inputs["ln2_g"], np.float32)[:L]),
        "c2": np.ascontiguousarray(np.asarray(inputs["ln2_b"], np.float32)[:L]),
    }
    in_maps = []
    for c in range(NCORES):
        row, b = c // 4, c % 4
        m = dict(shared)
        m["idx"] = np.ascontiguousarray(x[row])
        m["idxm"] = np.ascontiguousarray(x[row, b * SB:(b + 1) * SB])
        m["posTm"] = np.ascontiguousarray(posT[:, b * SB:(b + 1) * SB])
        in_maps.append(m)
    return in_maps


def run(inputs, n_layers=NLAYERS, trace=False):
    nc = _get_nc(n_layers, NCORES)
    in_maps = prep_in_maps(inputs, n_layers)
    res = run_bass_kernel_spmd(nc, in_maps, core_ids=list(range(NCORES)), trace=trace)
    full = np.zeros((B, S, D), np.float32)
    for c in range(NCORES):
        row, b = c // 4, c % 4
        full[row, b * SB:(b + 1) * SB, :] = res.results[c]["out"].T
    return full, res


def run_timed(inputs, n_layers=NLAYERS, iters=6):
    """Time the compiled NEFF with device-resident inputs (min over iters)."""
    import time
    import jax
    from jax.sharding import Mesh, PartitionSpec, NamedSharding
    from jax.experimental.shard_map import shard_map
    from concourse import mybir as _mybir
    from concourse.bass2jax import _bass_exec_p, install_neuronx_cc_hook, partition_id_tensor

    nc = _get_nc(n_layers, NCORES)
    in_maps = prep_in_maps(inputs, n_layers)
    install_neuronx_cc_hook()

    partition_name = nc.partition_id_tensor.name if nc.partition_id_tensor else None
    in_names, out_names, out_avals, zero_outs = [], [], [], []
    for alloc in nc.m.functions[0].allocations:
        if not isinstance(alloc, _mybir.MemoryLocationSet):
            continue
        name = alloc.memorylocations[0].name
        if alloc.kind == "ExternalInput":
            if name != partition_name:
                in_names.append(name)
        elif alloc.kind == "ExternalOutput":
            shape = tuple(alloc.tensor_shape)
            dtype = _mybir.dt.np(alloc.dtype)
            out_names.append(name)
            out_avals.append(jax.core.ShapedArray(shape, dtype))
            zero_outs.append(np.zeros(shape, dtype))
    n_params = len(in_names)
    n_outs = len(out_names)
    all_in_names = list(in_names) + list(out_names)
    if partition_name is not None:
        all_in_names.append(partition_name)

    def _body(*args):
        operands = list(args)
        if partition_name is not None:
            operands.append(partition_id_tensor())
        return tuple(_bass_exec_p.bind(
            *operands, out_avals=tuple(out_avals), in_names=tuple(all_in_names),
            out_names=tuple(out_names), lowering_input_output_aliases=(),
            sim_require_finite=True, sim_require_nnan=True, nc=nc))

    devices = jax.devices()[:NCORES]
    mesh = Mesh(np.asarray(devices), ("core",))
    nshard = NamedSharding(mesh, PartitionSpec("core"))
    donate = tuple(range(n_params, n_params + n_outs))
    fn = jax.jit(shard_map(_body, mesh=mesh,
                           in_specs=(PartitionSpec("core"),) * (n_params + n_outs),
                           out_specs=(PartitionSpec("core"),) * n_outs,
                           check_rep=False), donate_argnums=donate, keep_unused=True)
    concat_in = [np.concatenate([np.asarray(in_maps[c][nm]) for c in range(NCORES)], axis=0)
                 for nm in in_names]
    concat_zeros = [np.zeros((NCORES * z.shape[0], *z.shape[1:]), z.dtype) for z in zero_outs]
    dev_in = [jax.device_put(a, nshard) for a in concat_in]
    jax.block_until_ready(dev_in)

    def one_call():
        dz = [jax.device_put(z, nshard) for z in concat_zeros]
        jax.block_until_ready(dz)
        t0 = time.perf_counter()
        outs = fn(*dev_in, *dz)
        jax.block_until_ready(outs)
        return time.perf_counter() - t0, outs

    _, outs = one_call()  # compile + warm
    times = []
    for _ in range(iters):
        dt, outs = one_call()
        times.append(dt)
    full = np.zeros((B, S, D), np.float32)
    arr = np.asarray(outs[out_names.index("out")]).reshape(NCORES, D, SB)
    for c in range(NCORES):
        row, b = c // 4, c % 4
        full[row, b * SB:(b + 1) * SB, :] = arr[c].T
    return full, min(times), times


def run_async(inputs, n_layers=NLAYERS, nrep=16, iters=3):
    """Estimate device exec time via K pipelined async dispatches:
    slope of total time vs K removes the tunnel round-trip latency."""
    import time
    import jax
    from jax.sharding import Mesh, PartitionSpec, NamedSharding
    from jax.experimental.shard_map import shard_map
    from concourse import mybir as _mybir
    from concourse.bass2jax import _bass_exec_p, install_neuronx_cc_hook, partition_id_tensor

    nc = _get_nc(n_layers, NCORES)
    in_maps = prep_in_maps(inputs, n_layers)
    install_neuronx_cc_hook()

    partition_name = nc.partition_id_tensor.name if nc.partition_id_tensor else None
    in_names, out_names, out_avals, zero_outs = [], [], [], []
    for alloc in nc.m.functions[0].allocations:
        if not isinstance(alloc, _mybir.MemoryLocationSet):
            continue
        name = alloc.memorylocations[0].name
        if alloc.kind == "ExternalInput":
            if name != partition_name:
                in_names.append(name)
        elif alloc.kind == "ExternalOutput":
            shape = tuple(alloc.tensor_shape)
            dtype = _mybir.dt.np(alloc.dtype)
            out_names.append(name)
            out_avals.append(jax.core.ShapedArray(shape, dtype))
            zero_outs.append(np.zeros(shape, dtype))
    n_params = len(in_names)
    n_outs = len(out_names)
    all_in_names = list(in_names) + list(out_names)
    if partition_name is not None:
        all_in_names.append(partition_name)

    def _body(*args):
        operands = list(args)
        if partition_name is not None:
            operands.append(partition_id_tensor())
        return tuple(_bass_exec_p.bind(
            *operands, out_avals=tuple(out_avals), in_names=tuple(all_in_names),
            out_names=tuple(out_names), lowering_input_output_aliases=(),
            sim_require_finite=True, sim_require_nnan=True, nc=nc))

    devices = jax.devices()[:NCORES]
    mesh = Mesh(np.asarray(devices), ("core",))
    nshard = NamedSharding(mesh, PartitionSpec("core"))
    fn = jax.jit(shard_map(_body, mesh=mesh,
                           in_specs=(PartitionSpec("core"),) * (n_params + n_outs),
                           out_specs=(PartitionSpec("core"),) * n_outs,
                           check_rep=False), keep_unused=True)
    concat_in = [np.concatenate([np.asarray(in_maps[c][nm]) for c in range(NCORES)], axis=0)
                 for nm in in_names]
    concat_zeros = [np.zeros((NCORES * z.shape[0], *z.shape[1:]), z.dtype) for z in zero_outs]
    dev_args = [jax.device_put(a, nshard) for a in concat_in] +                [jax.device_put(z, nshard) for z in concat_zeros]
    jax.block_until_ready(dev_args)
    outs = fn(*dev_args)
    jax.block_until_ready(outs)

    def run_k(k):
        best = None
        for _ in range(iters):
            t0 = time.perf_counter()
            rs = [fn(*dev_args) for _ in range(k)]
            jax.block_until_ready(rs)
            dt = time.perf_counter() - t0
            best = dt if best is None else min(best, dt)
        return best

    t1 = run_k(1)
    tk = run_k(nrep)
    per_exec = (tk - t1) / (nrep - 1)
    full = np.zeros((B, S, D), np.float32)
    arr = np.asarray(outs[out_names.index("out")]).reshape(NCORES, D, SB)
    for c in range(NCORES):
        row, b = c // 4, c % 4
        full[row, b * SB:(b + 1) * SB, :] = arr[c].T
    return full, per_exec, {1: t1, nrep: tk}


def kernel(**inputs):
    full, _ = run(inputs)
    return full
